# revision 5
# baseline (speedup 1.0000x reference)
"""CenterPixelMSE on 8 TRN2 NeuronCores.

loss = mean_b (pred[b, 0, cy_b, cx_b] - target[b])^2
  pred: (512, 1, 256, 256) f32, target: (512,) f32, centers: (512, 2) i32

Strategy: pure data parallel over batch (64 per core). The loss touches one
pixel per batch element, so each core computes flat indices from its centers
shard on-device, gathers the 64 center pixels from its pred shard in HBM with
a single indirect DMA, squares the residual against target, and reduces the
64 partials with a 1-column matmul (folding in the 1/B mean scale). The host
sums the 8 per-core partials (the all-reduce of the sharding hint).
"""

import numpy as np

B, H, W = 512, 256, 256
NCORES = 8
BS = B // NCORES  # 64 batch elements per core

_NC_CACHE = {}


def _build_nc():
    import concourse.bass as bass
    import concourse.mybir as mybir
    from concourse import bacc
    from concourse.tile import TileContext

    # Bacc (not plain Bass): its compile() runs generate_event_semaphores,
    # which splits multi-sem waits to the TRN2 1-wait-per-instruction limit
    # (the kernel-tail drain needs this).
    nc = bacc.Bacc(debug=False)
    # pred shard is viewed as (BS*H*W, 1) so a flat element index gathers one pixel.
    pred = nc.dram_tensor("pred", [BS * H * W, 1], mybir.dt.float32, kind="ExternalInput")
    centers = nc.dram_tensor("centers", [BS, 2], mybir.dt.int32, kind="ExternalInput")
    target = nc.dram_tensor("target", [BS, 1], mybir.dt.float32, kind="ExternalInput")
    out = nc.dram_tensor("out", [1, 1], mybir.dt.float32, kind="ExternalOutput")

    # DVE TensorTensor/TensorScalar ISA slots fit only ONE sem wait, so every
    # compute instruction below is arranged to have at most one cross-engine
    # producer: cross-engine inputs are laundered through dedicated
    # tensor_copy instructions (each with exactly one wait), after which the
    # consumers only wait on the DVE self-semaphore.
    with TileContext(nc) as tc:
        with (
            tc.tile_pool(name="sbuf", bufs=1) as pool,
            tc.tile_pool(name="psum", bufs=1, space="PSUM") as psum_pool,
        ):
            ctile = pool.tile([BS, 2], mybir.dt.int32)
            nc.sync.dma_start(out=ctile[:], in_=centers[:])
            ttile = pool.tile([BS, 1], mybir.dt.float32)
            nc.sync.dma_start(out=ttile[:], in_=target[:])

            # flat_idx[b] = b*H*W + cy[b]*W + cx[b]
            base = pool.tile([BS, 1], mybir.dt.int32)
            nc.gpsimd.iota(base[:], [[0, 1]], channel_multiplier=H * W)
            base_c = pool.tile([BS, 1], mybir.dt.int32)
            nc.vector.tensor_copy(base_c[:], base[:])  # waits: Pool
            t_c = pool.tile([BS, 1], mybir.dt.float32)
            nc.vector.tensor_copy(t_c[:], ttile[:])  # waits: target DMA

            idx = pool.tile([BS, 1], mybir.dt.int32)
            # idx = cy*W + cx; waits: centers DMA
            nc.vector.scalar_tensor_tensor(
                out=idx[:],
                in0=ctile[:, 0:1],
                scalar=W,
                in1=ctile[:, 1:2],
                op0=mybir.AluOpType.mult,
                op1=mybir.AluOpType.add,
            )
            idx2 = pool.tile([BS, 1], mybir.dt.int32)
            nc.vector.tensor_add(idx2[:], idx[:], base_c[:])  # DVE-only deps

            gathered = pool.tile([BS, 1], mybir.dt.float32)
            nc.gpsimd.indirect_dma_start(
                out=gathered[:],
                out_offset=None,
                in_=pred[:],
                in_offset=bass.IndirectOffsetOnAxis(ap=idx2[:, 0:1], axis=0),
            )
            g_c = pool.tile([BS, 1], mybir.dt.float32)
            nc.vector.tensor_copy(g_c[:], gathered[:])  # waits: gather DMA

            diff = pool.tile([BS, 1], mybir.dt.float32)
            nc.vector.tensor_sub(diff[:], g_c[:], t_c[:])  # DVE-only deps
            sq = pool.tile([BS, 1], mybir.dt.float32)
            nc.vector.tensor_mul(sq[:], diff[:], diff[:])

            # Partition-axis reduction: [1,64]@[64,1] matmul, scaled by 1/B so
            # the host-side combine is a plain sum.
            scale = pool.tile([BS, 1], mybir.dt.float32)
            nc.vector.memset(scale[:], 1.0 / B)
            acc = psum_pool.tile([1, 1], mybir.dt.float32, space="PSUM")
            nc.tensor.matmul(out=acc[:], lhsT=sq[:], rhs=scale[:], start=True, stop=True)
            res = pool.tile([1, 1], mybir.dt.float32)
            nc.vector.tensor_copy(res[:], acc[:])
            nc.sync.dma_start(out=out[:], in_=res[:])
    nc.compile()
    return nc


def _shard_inputs(pred, target, centers):
    p = np.ascontiguousarray(pred, dtype=np.float32).reshape(NCORES, BS * H * W, 1)
    t = np.ascontiguousarray(target, dtype=np.float32).reshape(NCORES, BS, 1)
    c = np.ascontiguousarray(centers, dtype=np.int32).reshape(NCORES, BS, 2)
    return [
        {"pred": p[i], "centers": c[i], "target": t[i]} for i in range(NCORES)
    ]


def kernel(pred, target, centers, _debug_results=None, **run_kwargs):
    from concourse.bass_utils import run_bass_kernel_spmd

    if "nc" not in _NC_CACHE:
        _NC_CACHE["nc"] = _build_nc()
    nc = _NC_CACHE["nc"]

    in_maps = _shard_inputs(pred, target, centers)
    r = run_bass_kernel_spmd(nc, in_maps, core_ids=list(range(NCORES)), **run_kwargs)
    if _debug_results is not None:
        _debug_results.append(r)
    total = np.zeros((), dtype=np.float32)
    for m in r.results:
        total += m["out"].reshape(())
    return np.asarray(total, dtype=np.float32)
